# revision 22
# baseline (speedup 1.0000x reference)
"""Trainium2 Bass kernel for the controlled-U (CU) gate application.

Math: the reference builds U = P0 (x) I (x) ... + P1 (x) Mexp (x) I ...
with dim=2, wires=12, index=(0,1), control_state=(1,). This factors as

    U = diag(I_2048, Mexp (x) I_1024)        (4096 x 4096)

so U @ x is:
    out[0:2048]     = x[0:2048]                        (identity)
    out[2048:3072]  = c00 * x[2048:3072] + c01 * x[3072:4096]
    out[3072:4096]  = c10 * x[2048:3072] + c11 * x[3072:4096]

with [[c00, c01], [c10, c11]] = Mexp = expm(M - M^H), a 2x2 unitary
computed exactly on host (eigendecomposition of the 2x2 Hermitian
generator).

The identity block is pure data movement with zero compute, so it never
touches the device: the host writes x's top half straight into the
output buffer. Only the bottom half (the actual GEMM slice) runs on the
8 NeuronCores, as fp16 (the 2e-2 rel-err budget leaves ~40x margin).

Device strategy (row sharding over the 1024 bottom row-pairs; core d
owns pairs [128d, 128d+128)):
  - the 4 real input planes (re/im of the two coupled row blocks) are
    host-packed into one [128, 4224] fp16 tile: cols [0:128) hold the
    128x128 stationary, cols [128:4224) the data with partition index
    (plane, row%32) and free index (row//32, batch). In that layout the
    whole 4x4 real mixing matrix is the dense stationary W4 (x) I_32,
    and ONE matmul stream computes all four output planes
    simultaneously: 8 matmuls of N=512 fp16 columns.
  - the stationary rides in the first load chunk (one fewer DMA and a
    single-semaphore gate for the first matmul); loads alternate
    between the two HWDGE rings (SP / ACT) so the queue rows stream
    concurrently and per-DMA completion receipts overlap.
  - PSUM (all 8 banks) is evacuated to an fp16 SBUF tile with cast, at
    single-bank granularity alternating DVE / ACT so each chunk is
    converted as soon as its matmul lands; ACT's one-time activation
    table load is prefetched at t=0 behind a memset warmup.
  - host unpacks the [128, 4096] fp16 result into the complex64 output.

Per-core HBM traffic is 1 MiB in + 1 MiB out (vs 8 MiB for the naive
full-matrix fp32 version); the kernel is DMA-bound: ~9.5 us of the
measured time is the fixed Bass/Tile NEFF pre/postamble (semaphore
sweep) that even an empty kernel on this harness pays (measured
13.7 us for a minimal 1-load/1-copy/1-store kernel), ~4.5 us is the
load stream + completion receipts, and ~3 us the compute/store tail.
"""

import numpy as np

import concourse.bacc as bacc
import concourse.mybir as mybir
from concourse.tile import TileContext
from concourse.bass_utils import run_bass_kernel_spmd

# Problem geometry (hardcoded per the task contract).
D = 4096           # state dimension 2**12
B = 1024           # batch
NCORES = 8
P = 128            # SBUF partitions
PROWS = D // 4 // NCORES   # 128 bottom pair rows per core
F16 = mybir.dt.float16
F32 = mybir.dt.float32

NCOLS = 4 * B              # data cols of the packed tile: (row//32, batch)
XCOLS = P + NCOLS          # + leading stationary block
MM = 512                   # moving columns per matmul (= one PSUM bank fp32)
NMM = NCOLS // MM          # 8 matmuls
GRP = 2 * MM               # store granularity (2 PSUM banks)


def _build_nc() -> bacc.Bacc:
    """Build the per-core Bass/Tile program (identical on all 8 cores)."""
    # Bacc (not raw Bass): raw Bass trips walrus's per-instruction wait-slot
    # limit ("Too many sync wait commands") on the TileContext final drain.
    nc = bacc.Bacc("TRN2", enable_partition_id=False)

    x = nc.dram_tensor("x", [P, XCOLS], F16, kind="ExternalInput")
    o = nc.dram_tensor("o", [P, NCOLS], F16, kind="ExternalOutput")

    with TileContext(nc) as tc:
        with (
            tc.tile_pool(name="const", bufs=1) as const_pool,
            tc.tile_pool(name="io", bufs=1) as io_pool,
            tc.tile_pool(name="psum", bufs=7, space="PSUM") as psum_pool,
            tc.tile_pool(name="warmps", bufs=1, space="PSUM") as warm_psum,
        ):
            # Loads alternate between the two HWDGE rings (SP and ACT) so
            # the two queue rows stream concurrently and per-DMA completion
            # receipts overlap. The stationary rides in the first chunk.
            x_sb = io_pool.tile([P, XCOLS], F16, tag="x")
            bounds = [0, P + GRP, P + 2 * GRP, P + 3 * GRP, XCOLS]
            for c in range(4):
                cs = slice(bounds[c], bounds[c + 1])
                eng = nc.sync if c % 2 == 0 else nc.scalar
                eng.dma_start(x_sb[:, cs], x[:, cs])
            s_sb = x_sb[:, 0:P]

            # ACT warmup on a memset tile: triggers the one-time
            # ACT_TABLE_LOAD (~2.7us) behind the DMA ramp (after the
            # scalar-ring load issues) instead of stalling the first
            # PSUM evacuation.
            warm_sb = const_pool.tile([P, MM], F16, tag="warm")
            nc.gpsimd.memset(warm_sb[:], 0.0)
            warm_o = const_pool.tile([P, 16], F16, tag="warm_o")
            nc.scalar.copy(warm_o[:], warm_sb[:, 0:16])

            # PE warmup: ~3.4us of junk matmuls on the memset tile while the
            # loads stream, so the HAM clock gate lifts (1.2 -> 2.4 GHz)
            # before the real matmuls run. They fill the PE FIFO only until
            # the first load lands, writing a dedicated PSUM bank.
            warm_ps = warm_psum.tile([P, MM], F32, tag="wps")
            for _ in range(5):
                nc.tensor.matmul(warm_ps[:], warm_sb[:, 0:P], warm_sb[:],
                                 start=True, stop=True)

            # 8 matmuls (one per PSUM bank); evacuation at bank granularity
            # alternating DVE / ACT so each chunk is cast to fp16 as soon as
            # its matmul lands. Stores: 2-bank granularity on the SP ring,
            # except the final bank ships alone from the ACT ring right
            # behind its own evacuation (shortest possible tail).
            o_sb = io_pool.tile([P, NCOLS], F16, tag="o")
            for k in range(NMM):
                pt = psum_pool.tile([P, MM], F32, tag="ps")
                ks = slice(P + k * MM, P + (k + 1) * MM)
                nc.tensor.matmul(pt[:], s_sb, x_sb[:, ks],
                                 start=True, stop=True)
                os_ = o_sb[:, k * MM : (k + 1) * MM]
                if k % 2 == 1:
                    nc.scalar.copy(os_, pt[:])
                else:
                    nc.vector.tensor_copy(os_, pt[:])
                if k % 2 == 1 and k < NMM - 1:
                    gs = slice((k - 1) * MM, (k + 1) * MM)
                    nc.sync.dma_start(o[:, gs], o_sb[:, gs])
                elif k == NMM - 2:
                    # last two chunks ship individually: c6 from the SP ring
                    # behind its DVE evac, c7 from the ACT ring right behind
                    # its own ACT evac (short single-chunk tail, light final
                    # completion receipt).
                    gs = slice(k * MM, (k + 1) * MM)
                    nc.sync.dma_start(o[:, gs], o_sb[:, gs])
                elif k == NMM - 1:
                    gs = slice(k * MM, (k + 1) * MM)
                    nc.scalar.dma_start(o[:, gs], o_sb[:, gs])

    nc.finalize()
    return nc


_NC_CACHE = None


def _get_nc() -> bacc.Bacc:
    global _NC_CACHE
    if _NC_CACHE is None:
        _NC_CACHE = _build_nc()
    return _NC_CACHE


def _mexp(M_re: np.ndarray, M_im: np.ndarray) -> np.ndarray:
    """Host-side 2x2 expm of the anti-Hermitian generator (exact)."""
    M = M_re.astype(np.float64) + 1j * M_im.astype(np.float64)
    A = M - M.conj().T          # anti-Hermitian
    H = -1j * A                 # Hermitian
    w, V = np.linalg.eigh(H)
    return V @ np.diag(np.exp(1j * w)) @ V.conj().T   # expm(A)


def _stationary(Mexp: np.ndarray) -> np.ndarray:
    """kron(W4.T, I_32) fp16 stationary: out planes = W4 @ in planes."""
    a, b = Mexp.real, Mexp.imag
    W4 = np.array([
        [a[0, 0], -b[0, 0], a[0, 1], -b[0, 1]],   # o1.re
        [b[0, 0],  a[0, 0], b[0, 1],  a[0, 1]],   # o1.im
        [a[1, 0], -b[1, 0], a[1, 1], -b[1, 1]],   # o2.re
        [b[1, 0],  a[1, 0], b[1, 1],  a[1, 1]],   # o2.im
    ])
    return np.kron(W4.T, np.eye(32)).astype(np.float16)


def _host_prep(M_re, M_im, x_re, x_im):
    s = _stationary(_mexp(M_re, M_im))
    in_maps = []
    for d in range(NCORES):
        b1 = D // 2 + d * PROWS
        b2 = 3 * D // 4 + d * PROWS
        planes = (x_re[b1 : b1 + PROWS], x_im[b1 : b1 + PROWS],
                  x_re[b2 : b2 + PROWS], x_im[b2 : b2 + PROWS])
        X = np.empty((P, XCOLS), dtype=np.float16)
        X[:, :P] = s
        for p, pl in enumerate(planes):
            # [128, 1024] -> partition (plane, row%32), free (row//32, b)
            X[p * 32 : (p + 1) * 32, P:] = (
                pl.reshape(4, 32, B).transpose(1, 0, 2).reshape(32, NCOLS))
        in_maps.append({"x": X})
    return in_maps


def _assemble(results, x_re, x_im) -> np.ndarray:
    out = np.empty((D, B), dtype=np.complex64)
    out.real[: D // 2] = x_re[: D // 2]
    out.imag[: D // 2] = x_im[: D // 2]
    for d, r in enumerate(results):
        # [128, 4096] -> (q, row%32, row//32, b) -> (q, row, b)
        Q = (r["o"].reshape(4, 32, 4, B).transpose(0, 2, 1, 3)
             .reshape(4, PROWS, B))
        b1 = D // 2 + d * PROWS
        b2 = 3 * D // 4 + d * PROWS
        out.real[b1 : b1 + PROWS] = Q[0]
        out.imag[b1 : b1 + PROWS] = Q[1]
        out.real[b2 : b2 + PROWS] = Q[2]
        out.imag[b2 : b2 + PROWS] = Q[3]
    return out


def kernel(M_re, M_im, x_re, x_im) -> np.ndarray:
    M_re = np.asarray(M_re, dtype=np.float32)
    M_im = np.asarray(M_im, dtype=np.float32)
    x_re = np.ascontiguousarray(x_re, dtype=np.float32)
    x_im = np.ascontiguousarray(x_im, dtype=np.float32)

    in_maps = _host_prep(M_re, M_im, x_re, x_im)
    nc = _get_nc()
    res = run_bass_kernel_spmd(nc, in_maps, core_ids=list(range(NCORES)))
    return _assemble(res.results, x_re, x_im)


# revision 23
# speedup vs baseline: 1.0599x; 1.0599x over previous
"""Trainium2 Bass kernel for the controlled-U (CU) gate application.

Math: the reference builds U = P0 (x) I (x) ... + P1 (x) Mexp (x) I ...
with dim=2, wires=12, index=(0,1), control_state=(1,). This factors as

    U = diag(I_2048, Mexp (x) I_1024)        (4096 x 4096)

so U @ x is:
    out[0:2048]     = x[0:2048]                        (identity)
    out[2048:3072]  = c00 * x[2048:3072] + c01 * x[3072:4096]
    out[3072:4096]  = c10 * x[2048:3072] + c11 * x[3072:4096]

with [[c00, c01], [c10, c11]] = Mexp = expm(M - M^H), a 2x2 unitary
computed exactly on host (eigendecomposition of the 2x2 Hermitian
generator).

The identity block is pure data movement with zero compute, so it never
touches the device: the host writes x's top half straight into the
output buffer. Only the bottom half (the actual GEMM slice) runs on the
8 NeuronCores, as fp16 (the 2e-2 rel-err budget leaves ~40x margin).

Device strategy (row sharding over the 1024 bottom row-pairs; core d
owns pairs [128d, 128d+128)):
  - the 4 real input planes (re/im of the two coupled row blocks) are
    host-packed into one [128, 4224] fp16 tile: cols [0:128) hold the
    128x128 stationary, cols [128:4224) the data with partition index
    (plane, row%32) and free index (row//32, batch). In that layout the
    whole 4x4 real mixing matrix is the dense stationary W4 (x) I_32,
    and ONE matmul stream computes all four output planes
    simultaneously: 8 matmuls of N=512 fp16 columns.
  - the stationary rides in the first load chunk (one fewer DMA and a
    single-semaphore gate for the first matmul); loads alternate
    between the two HWDGE rings (SP / ACT) so the queue rows stream
    concurrently and per-DMA completion receipts overlap.
  - PSUM (7 banks + 1 warmup bank) is evacuated to an fp16 SBUF tile
    with cast, at single-bank granularity alternating DVE / ACT so each
    chunk is converted as soon as its matmul lands; ACT's one-time
    activation table load is prefetched at t=0 behind a memset warmup,
    and ~2us of junk matmuls on the same memset tile lift the PE HAM
    clock gate (1.2 -> 2.4 GHz) before the real matmuls run.
  - the last two output chunks ship as separate small stores (the final
    one from the ACT ring right behind its own evacuation) to shorten
    the store tail.
  - host unpacks the [128, 4096] fp16 result into the complex64 output.

Per-core HBM traffic is 1 MiB in + 1 MiB out (vs 8 MiB for the naive
full-matrix fp32 version); the kernel is DMA-bound: ~9.5 us of the
measured time is the fixed Bass/Tile NEFF pre/postamble (semaphore
sweep) that even an empty kernel on this harness pays (measured
13.7 us for a minimal 1-load/1-copy/1-store kernel), ~4.5 us is the
load stream + completion receipts, and ~3 us the compute/store tail.
"""

import numpy as np

import concourse.bacc as bacc
import concourse.mybir as mybir
from concourse.tile import TileContext
from concourse.bass_utils import run_bass_kernel_spmd

# Problem geometry (hardcoded per the task contract).
D = 4096           # state dimension 2**12
B = 1024           # batch
NCORES = 8
P = 128            # SBUF partitions
PROWS = D // 4 // NCORES   # 128 bottom pair rows per core
F16 = mybir.dt.float16
F32 = mybir.dt.float32

NCOLS = 4 * B              # data cols of the packed tile: (row//32, batch)
XCOLS = P + NCOLS          # + leading stationary block
MM = 512                   # moving columns per matmul (= one PSUM bank fp32)
NMM = NCOLS // MM          # 8 matmuls
GRP = 2 * MM               # store granularity (2 PSUM banks)


def _build_nc() -> bacc.Bacc:
    """Build the per-core Bass/Tile program (identical on all 8 cores)."""
    # Bacc (not raw Bass): raw Bass trips walrus's per-instruction wait-slot
    # limit ("Too many sync wait commands") on the TileContext final drain.
    nc = bacc.Bacc("TRN2", enable_partition_id=False)

    x = nc.dram_tensor("x", [P, XCOLS], F16, kind="ExternalInput")
    o = nc.dram_tensor("o", [P, NCOLS], F16, kind="ExternalOutput")

    with TileContext(nc) as tc:
        with (
            tc.tile_pool(name="const", bufs=1) as const_pool,
            tc.tile_pool(name="io", bufs=1) as io_pool,
            tc.tile_pool(name="psum", bufs=7, space="PSUM") as psum_pool,
            tc.tile_pool(name="warmps", bufs=1, space="PSUM") as warm_psum,
        ):
            # Loads alternate between the two HWDGE rings (SP and ACT) so
            # the two queue rows stream concurrently and per-DMA completion
            # receipts overlap. The stationary rides in the first chunk.
            x_sb = io_pool.tile([P, XCOLS], F16, tag="x")
            bounds = [0, P + GRP, P + 2 * GRP, P + 3 * GRP, XCOLS]
            for c in range(4):
                cs = slice(bounds[c], bounds[c + 1])
                eng = nc.sync if c % 2 == 0 else nc.scalar
                eng.dma_start(x_sb[:, cs], x[:, cs])
            s_sb = x_sb[:, 0:P]

            # ACT warmup on a memset tile: triggers the one-time
            # ACT_TABLE_LOAD (~2.7us) behind the DMA ramp (after the
            # scalar-ring load issues) instead of stalling the first
            # PSUM evacuation.
            warm_sb = const_pool.tile([P, MM], F16, tag="warm")
            nc.gpsimd.memset(warm_sb[:], 0.0)
            warm_o = const_pool.tile([P, 16], F16, tag="warm_o")
            nc.scalar.copy(warm_o[:], warm_sb[:, 0:16])

            # PE warmup: ~3.4us of junk matmuls on the memset tile while the
            # loads stream, so the HAM clock gate lifts (1.2 -> 2.4 GHz)
            # before the real matmuls run. They fill the PE FIFO only until
            # the first load lands, writing a dedicated PSUM bank.
            warm_ps = warm_psum.tile([P, MM], F32, tag="wps")
            for _ in range(5):
                nc.tensor.matmul(warm_ps[:], warm_sb[:, 0:P], warm_sb[:],
                                 start=True, stop=True)

            # 8 matmuls (one per PSUM bank); evacuation at bank granularity
            # alternating DVE / ACT so each chunk is cast to fp16 as soon as
            # its matmul lands. Stores: 2-bank granularity on the SP ring,
            # except the final bank ships alone from the ACT ring right
            # behind its own evacuation (shortest possible tail).
            o_sb = io_pool.tile([P, NCOLS], F16, tag="o")
            for k in range(NMM):
                pt = psum_pool.tile([P, MM], F32, tag="ps")
                ks = slice(P + k * MM, P + (k + 1) * MM)
                nc.tensor.matmul(pt[:], s_sb, x_sb[:, ks],
                                 start=True, stop=True)
                os_ = o_sb[:, k * MM : (k + 1) * MM]
                if k % 2 == 1:
                    nc.scalar.copy(os_, pt[:])
                else:
                    nc.vector.tensor_copy(os_, pt[:])
                if k % 2 == 1 and k < NMM - 1:
                    gs = slice((k - 1) * MM, (k + 1) * MM)
                    nc.sync.dma_start(o[:, gs], o_sb[:, gs])
                elif k == NMM - 2:
                    # last two chunks ship individually: c6 from the SP ring
                    # behind its DVE evac, c7 from the ACT ring right behind
                    # its own ACT evac (short single-chunk tail, light final
                    # completion receipt).
                    gs = slice(k * MM, (k + 1) * MM)
                    nc.sync.dma_start(o[:, gs], o_sb[:, gs])
                elif k == NMM - 1:
                    gs = slice(k * MM, (k + 1) * MM)
                    nc.scalar.dma_start(o[:, gs], o_sb[:, gs])

    nc.finalize()
    return nc


_NC_CACHE = None


def _get_nc() -> bacc.Bacc:
    global _NC_CACHE
    if _NC_CACHE is None:
        _NC_CACHE = _build_nc()
    return _NC_CACHE


def _mexp(M_re: np.ndarray, M_im: np.ndarray) -> np.ndarray:
    """Host-side 2x2 expm of the anti-Hermitian generator (exact)."""
    M = M_re.astype(np.float64) + 1j * M_im.astype(np.float64)
    A = M - M.conj().T          # anti-Hermitian
    H = -1j * A                 # Hermitian
    w, V = np.linalg.eigh(H)
    return V @ np.diag(np.exp(1j * w)) @ V.conj().T   # expm(A)


def _stationary(Mexp: np.ndarray) -> np.ndarray:
    """kron(W4.T, I_32) fp16 stationary: out planes = W4 @ in planes."""
    a, b = Mexp.real, Mexp.imag
    W4 = np.array([
        [a[0, 0], -b[0, 0], a[0, 1], -b[0, 1]],   # o1.re
        [b[0, 0],  a[0, 0], b[0, 1],  a[0, 1]],   # o1.im
        [a[1, 0], -b[1, 0], a[1, 1], -b[1, 1]],   # o2.re
        [b[1, 0],  a[1, 0], b[1, 1],  a[1, 1]],   # o2.im
    ])
    return np.kron(W4.T, np.eye(32)).astype(np.float16)


def _host_prep(M_re, M_im, x_re, x_im):
    s = _stationary(_mexp(M_re, M_im))
    in_maps = []
    for d in range(NCORES):
        b1 = D // 2 + d * PROWS
        b2 = 3 * D // 4 + d * PROWS
        planes = (x_re[b1 : b1 + PROWS], x_im[b1 : b1 + PROWS],
                  x_re[b2 : b2 + PROWS], x_im[b2 : b2 + PROWS])
        X = np.empty((P, XCOLS), dtype=np.float16)
        X[:, :P] = s
        for p, pl in enumerate(planes):
            # [128, 1024] -> partition (plane, row%32), free (row//32, b)
            X[p * 32 : (p + 1) * 32, P:] = (
                pl.reshape(4, 32, B).transpose(1, 0, 2).reshape(32, NCOLS))
        in_maps.append({"x": X})
    return in_maps


def _assemble(results, x_re, x_im) -> np.ndarray:
    out = np.empty((D, B), dtype=np.complex64)
    out.real[: D // 2] = x_re[: D // 2]
    out.imag[: D // 2] = x_im[: D // 2]
    for d, r in enumerate(results):
        # [128, 4096] -> (q, row%32, row//32, b) -> (q, row, b)
        Q = (r["o"].reshape(4, 32, 4, B).transpose(0, 2, 1, 3)
             .reshape(4, PROWS, B))
        b1 = D // 2 + d * PROWS
        b2 = 3 * D // 4 + d * PROWS
        out.real[b1 : b1 + PROWS] = Q[0]
        out.imag[b1 : b1 + PROWS] = Q[1]
        out.real[b2 : b2 + PROWS] = Q[2]
        out.imag[b2 : b2 + PROWS] = Q[3]
    return out


def kernel(M_re, M_im, x_re, x_im) -> np.ndarray:
    M_re = np.asarray(M_re, dtype=np.float32)
    M_im = np.asarray(M_im, dtype=np.float32)
    x_re = np.ascontiguousarray(x_re, dtype=np.float32)
    x_im = np.ascontiguousarray(x_im, dtype=np.float32)

    in_maps = _host_prep(M_re, M_im, x_re, x_im)
    nc = _get_nc()
    res = run_bass_kernel_spmd(nc, in_maps, core_ids=list(range(NCORES)))
    return _assemble(res.results, x_re, x_im)


# revision 25
# speedup vs baseline: 1.1045x; 1.0421x over previous
"""Trainium2 Bass kernel for the controlled-U (CU) gate application.

Math: the reference builds U = P0 (x) I (x) ... + P1 (x) Mexp (x) I ...
with dim=2, wires=12, index=(0,1), control_state=(1,). This factors as

    U = diag(I_2048, Mexp (x) I_1024)        (4096 x 4096)

so U @ x is:
    out[0:2048]     = x[0:2048]                        (identity)
    out[2048:3072]  = c00 * x[2048:3072] + c01 * x[3072:4096]
    out[3072:4096]  = c10 * x[2048:3072] + c11 * x[3072:4096]

with [[c00, c01], [c10, c11]] = Mexp = expm(M - M^H), a 2x2 unitary
computed exactly on host (eigendecomposition of the 2x2 Hermitian
generator).

The identity block is pure data movement with zero compute, so it never
touches the device: the host writes x's top half straight into the
output buffer. Only the bottom half (the actual GEMM slice) runs on the
8 NeuronCores, as fp16 (the 2e-2 rel-err budget leaves ~40x margin).

Device strategy (row sharding over the 1024 bottom row-pairs; core d
owns pairs [128d, 128d+128)):
  - the 4 real input planes (re/im of the two coupled row blocks) are
    host-packed into one [128, 4224] fp16 tile: cols [0:128) hold the
    128x128 stationary, cols [128:4224) the data with partition index
    (plane, row%32) and free index (row//32, batch). In that layout the
    whole 4x4 real mixing matrix is the dense stationary W4 (x) I_32,
    and ONE matmul stream computes all four output planes
    simultaneously: 8 matmuls of N=512 fp16 columns.
  - the stationary rides in the first load chunk (one fewer DMA and a
    single-semaphore gate for the first matmul); loads alternate
    between the two HWDGE rings (SP / ACT) so the queue rows stream
    concurrently and per-DMA completion receipts overlap.
  - PSUM (all 8 banks) is evacuated to an fp16 SBUF tile with cast, at
    single-bank granularity alternating DVE / ACT so each chunk is
    converted as soon as its matmul lands; ACT's one-time activation
    table load is prefetched at t=0 behind a memset warmup. (A PE HAM
    pre-warm via junk matmuls was tried and reverted: halving the real
    matmul chain 427->216 ns never beat this config -- the extra PE
    activity during the DMA phase correlates with whole-chip slowdowns,
    consistent with the P0 power downclock.)
  - host unpacks the [128, 4096] fp16 result into the complex64 output.

Per-core HBM traffic is 1 MiB in + 1 MiB out (vs 8 MiB for the naive
full-matrix fp32 version); the kernel is DMA-bound: ~9.5 us of the
measured time is the fixed Bass/Tile NEFF pre/postamble (semaphore
sweep) that even an empty kernel on this harness pays (measured
13.7 us for a minimal 1-load/1-copy/1-store kernel), ~4.5 us is the
load stream + completion receipts, and ~3 us the compute/store tail.
"""

import numpy as np

import concourse.bacc as bacc
import concourse.mybir as mybir
from concourse.tile import TileContext
from concourse.bass_utils import run_bass_kernel_spmd

# Problem geometry (hardcoded per the task contract).
D = 4096           # state dimension 2**12
B = 1024           # batch
NCORES = 8
P = 128            # SBUF partitions
PROWS = D // 4 // NCORES   # 128 bottom pair rows per core
F16 = mybir.dt.float16
F32 = mybir.dt.float32

NCOLS = 4 * B              # data cols of the packed tile: (row//32, batch)
XCOLS = P + NCOLS          # + leading stationary block
MM = 512                   # moving columns per matmul (= one PSUM bank fp32)
NMM = NCOLS // MM          # 8 matmuls
GRP = 2 * MM               # store granularity (2 PSUM banks)


def _build_nc() -> bacc.Bacc:
    """Build the per-core Bass/Tile program (identical on all 8 cores)."""
    # Bacc (not raw Bass): raw Bass trips walrus's per-instruction wait-slot
    # limit ("Too many sync wait commands") on the TileContext final drain.
    nc = bacc.Bacc("TRN2", enable_partition_id=False)

    x = nc.dram_tensor("x", [P, XCOLS], F16, kind="ExternalInput")
    o = nc.dram_tensor("o", [P, NCOLS], F16, kind="ExternalOutput")

    with TileContext(nc) as tc:
        with (
            tc.tile_pool(name="const", bufs=1) as const_pool,
            tc.tile_pool(name="io", bufs=1) as io_pool,
            tc.tile_pool(name="psum", bufs=8, space="PSUM") as psum_pool,
        ):
            # Loads alternate between the two HWDGE rings (SP and ACT) so
            # the two queue rows stream concurrently and per-DMA completion
            # receipts overlap. The stationary rides in the first chunk.
            x_sb = io_pool.tile([P, XCOLS], F16, tag="x")
            bounds = [0, P + GRP, P + 2 * GRP, P + 3 * GRP, XCOLS]
            for c in range(4):
                cs = slice(bounds[c], bounds[c + 1])
                eng = nc.sync if c % 2 == 0 else nc.scalar
                eng.dma_start(x_sb[:, cs], x[:, cs])
            s_sb = x_sb[:, 0:P]

            # ACT warmup on a memset tile: triggers the one-time
            # ACT_TABLE_LOAD (~2.7us) behind the DMA ramp (after the
            # scalar-ring load issues) instead of stalling the first
            # PSUM evacuation.
            warm_sb = const_pool.tile([P, 16], F16, tag="warm")
            nc.gpsimd.memset(warm_sb[:], 0.0)
            warm_o = const_pool.tile([P, 16], F16, tag="warm_o")
            nc.scalar.copy(warm_o[:], warm_sb[:])

            # 8 matmuls (one per PSUM bank); evacuation at bank granularity
            # alternating DVE / ACT so each chunk is cast to fp16 as soon as
            # its matmul lands. Stores: 2-bank granularity on the SP ring.
            o_sb = io_pool.tile([P, NCOLS], F16, tag="o")
            for k in range(NMM):
                pt = psum_pool.tile([P, MM], F32, tag="ps")
                ks = slice(P + k * MM, P + (k + 1) * MM)
                nc.tensor.matmul(pt[:], s_sb, x_sb[:, ks],
                                 start=True, stop=True)
                os_ = o_sb[:, k * MM : (k + 1) * MM]
                if k % 2 == 1:
                    nc.scalar.copy(os_, pt[:])
                else:
                    nc.vector.tensor_copy(os_, pt[:])
                if k % 2 == 1:
                    gs = slice((k - 1) * MM, (k + 1) * MM)
                    nc.sync.dma_start(o[:, gs], o_sb[:, gs])

    nc.finalize()
    return nc


_NC_CACHE = None


def _get_nc() -> bacc.Bacc:
    global _NC_CACHE
    if _NC_CACHE is None:
        _NC_CACHE = _build_nc()
    return _NC_CACHE


def _mexp(M_re: np.ndarray, M_im: np.ndarray) -> np.ndarray:
    """Host-side 2x2 expm of the anti-Hermitian generator (exact)."""
    M = M_re.astype(np.float64) + 1j * M_im.astype(np.float64)
    A = M - M.conj().T          # anti-Hermitian
    H = -1j * A                 # Hermitian
    w, V = np.linalg.eigh(H)
    return V @ np.diag(np.exp(1j * w)) @ V.conj().T   # expm(A)


def _stationary(Mexp: np.ndarray) -> np.ndarray:
    """kron(W4.T, I_32) fp16 stationary: out planes = W4 @ in planes."""
    a, b = Mexp.real, Mexp.imag
    W4 = np.array([
        [a[0, 0], -b[0, 0], a[0, 1], -b[0, 1]],   # o1.re
        [b[0, 0],  a[0, 0], b[0, 1],  a[0, 1]],   # o1.im
        [a[1, 0], -b[1, 0], a[1, 1], -b[1, 1]],   # o2.re
        [b[1, 0],  a[1, 0], b[1, 1],  a[1, 1]],   # o2.im
    ])
    return np.kron(W4.T, np.eye(32)).astype(np.float16)


def _host_prep(M_re, M_im, x_re, x_im):
    s = _stationary(_mexp(M_re, M_im))
    in_maps = []
    for d in range(NCORES):
        b1 = D // 2 + d * PROWS
        b2 = 3 * D // 4 + d * PROWS
        planes = (x_re[b1 : b1 + PROWS], x_im[b1 : b1 + PROWS],
                  x_re[b2 : b2 + PROWS], x_im[b2 : b2 + PROWS])
        X = np.empty((P, XCOLS), dtype=np.float16)
        X[:, :P] = s
        for p, pl in enumerate(planes):
            # [128, 1024] -> partition (plane, row%32), free (row//32, b)
            X[p * 32 : (p + 1) * 32, P:] = (
                pl.reshape(4, 32, B).transpose(1, 0, 2).reshape(32, NCOLS))
        in_maps.append({"x": X})
    return in_maps


def _assemble(results, x_re, x_im) -> np.ndarray:
    out = np.empty((D, B), dtype=np.complex64)
    out.real[: D // 2] = x_re[: D // 2]
    out.imag[: D // 2] = x_im[: D // 2]
    for d, r in enumerate(results):
        # [128, 4096] -> (q, row%32, row//32, b) -> (q, row, b)
        Q = (r["o"].reshape(4, 32, 4, B).transpose(0, 2, 1, 3)
             .reshape(4, PROWS, B))
        b1 = D // 2 + d * PROWS
        b2 = 3 * D // 4 + d * PROWS
        out.real[b1 : b1 + PROWS] = Q[0]
        out.imag[b1 : b1 + PROWS] = Q[1]
        out.real[b2 : b2 + PROWS] = Q[2]
        out.imag[b2 : b2 + PROWS] = Q[3]
    return out


def kernel(M_re, M_im, x_re, x_im) -> np.ndarray:
    M_re = np.asarray(M_re, dtype=np.float32)
    M_im = np.asarray(M_im, dtype=np.float32)
    x_re = np.ascontiguousarray(x_re, dtype=np.float32)
    x_im = np.ascontiguousarray(x_im, dtype=np.float32)

    in_maps = _host_prep(M_re, M_im, x_re, x_im)
    nc = _get_nc()
    res = run_bass_kernel_spmd(nc, in_maps, core_ids=list(range(NCORES)))
    return _assemble(res.results, x_re, x_im)
